# revision 1
# baseline (speedup 1.0000x reference)
"""Trainium2 Bass kernel for nn_BasisOrbitalBackflow.

Math (reference collapses the N x N pair pooling):
    chi[b,i,mu]   = hermite_prod(ri[b,i], mu) * exp(-0.5 sigma_mu^2 |ri[b,i]|^2)
    S[b,mu]       = sum_i chi[b,i,mu]
    A[b,i,p]      = S[b,p] - chi[b,i,p]
    out[b,i,o]    = sum_{p,q} A[b,i,p] chi[b,i,q] C[p,q,o] / (N-1)

Device strategy (pure data parallel over batch, 8 cores, 256 batches each):
    * basis chi built row-major [112 partitions, 32 tiles * 20 mu] on DVE/ACT
      (Hermite polys rescaled by exact powers of two; scale folded into C)
    * PE transposes basis into a packed layout [(g:4)(mu:32)=128p, (t:8)(128)]
    * S = free-dim segment-reduce over i (DVE), A = S_bcast - B (DVE)
    * C (permuted/scaled) decomposed EXACTLY as rank-280 sum of outer
      products via mode-3 SVD + per-slice SVD:  C[p,q,o] = sum_m U[p,m] V[q,m] Z[m,o]
      -> out^T = Z^T @ ((U^T A^T) * (V^T B^T)),  rho padded 280 -> 384 (3 chunks of 128)
    * the two rank projections + output projection are float32r TensorE
      matmuls with N >= 256 (full 1 col/cycle rate); the single elementwise
      product runs on DVE in [128p, rows] layout
    * output produced as out^T [14, 3584] per core; host transposes/reshapes
"""

import itertools
import numpy as np

N_MAX = 3
SDIM = 3
N_PART = 14
BATCH = 2048
NB = 20
N_CORES = 8
BC = BATCH // N_CORES          # 256 batches per core
R = BC * N_PART                # 3584 rows per core
P = 112                        # rows per tile (8 batches)
T = R // P                     # 32 tiles
G = 4                          # transposed-layout groups (8 tiles each)
TG = T // G                    # tiles per group
RHO = 280                      # exact CP rank
RHO_PAD = 384                  # 3 chunks of 128
NCH = 3
MS = 32                        # mu slot stride (20 real + 12 pad)
WARMUP_MM = 20                 # junk matmuls to warm the PE clock gate
WARMUP2_MM = 10                # gap-filler matmuls during the S/A phase

# ---------------------------------------------------------------------------
# host-side constant construction
# ---------------------------------------------------------------------------

# reference mu ordering (sorted by |n|, stable)
_NS_REF = [tuple(n) for n in sorted(
    (n for n in itertools.product(range(N_MAX + 1), repeat=SDIM) if sum(n) <= N_MAX),
    key=sum)]

# our mu ordering, chosen so the product assembly uses few strided DVE ops.
# pairs (n1, n2) in assembly order:
_PAIRS = [(0, 0), (0, 1), (0, 2), (0, 3), (1, 0), (1, 1), (1, 2), (2, 0), (2, 1), (3, 0)]
_NS_OURS = ([(0,) + pr for pr in _PAIRS]
            + [(1,) + _PAIRS[k] for k in (0, 1, 2, 4, 5, 7)]
            + [(2,) + _PAIRS[k] for k in (0, 1, 4)]
            + [(3, 0, 0)])
assert sorted(_NS_OURS) == sorted(_NS_REF) and len(_NS_OURS) == NB
_PERM = np.array([_NS_REF.index(n) for n in _NS_OURS], dtype=np.int64)  # ours -> ref
_ABS_N = np.array([sum(n) for n in _NS_OURS], dtype=np.float64)


def _decompose(coeff, sigma):
    """Build all device constants from the (400,14) coeff and (20,) sigma."""
    C = np.asarray(coeff, dtype=np.float64).reshape(NB, NB, N_PART)
    # permute to our mu order and fold 2^{|n|} Hermite rescale + 1/(N-1)
    C = C[np.ix_(_PERM, _PERM)]
    scale = 2.0 ** _ABS_N
    C = C * scale[:, None, None] * scale[None, :, None] / (N_PART - 1)

    # exact rank-280 decomposition: mode-3 SVD, then SVD of each 20x20 slice
    C3 = C.reshape(NB * NB, N_PART)
    Uo, so, Vto = np.linalg.svd(C3, full_matrices=False)   # (400,14),(14,),(14,14)
    Us, Vs, Zs = [], [], []
    for k in range(N_PART):
        Wk = (Uo[:, k] * so[k]).reshape(NB, NB)
        uu, ss, vvt = np.linalg.svd(Wk)
        rt = np.sqrt(ss)
        for j in range(NB):
            Us.append(uu[:, j] * rt[j])
            Vs.append(vvt[j] * rt[j])
            Zs.append(Vto[k])
    U = np.stack(Us, 1)            # (20, 280)
    V = np.stack(Vs, 1)            # (20, 280)
    Z = np.stack(Zs, 0)            # (280, 14)

    # weights live at the same partition offsets as the packed A^T/B^T blocks
    Upad = np.zeros((128, RHO_PAD)); Vpad = np.zeros((128, RHO_PAD))
    for g in range(G):
        Upad[32 * g:32 * g + NB, :RHO] = U
        Vpad[32 * g:32 * g + NB, :RHO] = V
    Zpad = np.zeros((RHO_PAD, N_PART)); Zpad[:RHO] = Z
    # device z layout: [128, (c:3)(o:14)]
    zdev = np.zeros((128, NCH * N_PART))
    for c in range(NCH):
        zdev[:, c * N_PART:(c + 1) * N_PART] = Zpad[c * 128:(c + 1) * 128]

    sig = np.asarray(sigma, dtype=np.float64)
    s2 = -0.5 * (sig[_PERM] ** 2)                       # per-mu, our order
    s2c = np.broadcast_to(s2, (P, NB)).copy()           # replicated to partitions

    return (Upad.astype(np.float32), Vpad.astype(np.float32),
            zdev.astype(np.float32), s2c.astype(np.float32))


# ---------------------------------------------------------------------------
# device program
# ---------------------------------------------------------------------------

_PROGRAM = None


def _build_program():
    import concourse.bacc as bacc
    import concourse.tile as tile
    import concourse.mybir as mybir
    from concourse._compat import axon_active

    dt = mybir.dt
    f32 = dt.float32
    f32r = dt.float32r
    Alu = mybir.AluOpType
    ActF = mybir.ActivationFunctionType

    nc = bacc.Bacc(
        "TRN2",
        target_bir_lowering=False,
        debug=not axon_active(),
        num_devices=N_CORES,
    )

    x_d = nc.dram_tensor("x", [P, T * SDIM], f32, kind="ExternalInput")
    s2c_d = nc.dram_tensor("s2c", [P, NB], f32, kind="ExternalInput")
    id_d = nc.dram_tensor("ident", [P, P], f32, kind="ExternalInput")
    u_d = nc.dram_tensor("u", [128, RHO_PAD], f32, kind="ExternalInput")
    v_d = nc.dram_tensor("v", [128, RHO_PAD], f32, kind="ExternalInput")
    z_d = nc.dram_tensor("z", [128, NCH * N_PART], f32, kind="ExternalInput")
    out_d = nc.dram_tensor("out_t", [N_PART, R], f32, kind="ExternalOutput")

    with tile.TileContext(nc) as tc:
        with (
            tc.tile_pool(name="sb", bufs=1) as sb,
            tc.tile_pool(name="ps", bufs=4, space="PSUM") as ps,
        ):
            htab = sb.tile([P, T * SDIM * 4], f32, tag="htab")
            x2 = sb.tile([P, T * SDIM], f32, tag="x2")
            rho_t = sb.tile([P, T], f32, tag="rho")
            s2c = sb.tile([P, NB], f32, tag="s2c")
            ident = sb.tile([P, P], f32, tag="ident")
            u_st = sb.tile([128, RHO_PAD], f32, tag="u_st")
            v_st = sb.tile([128, RHO_PAD], f32, tag="v_st")
            z_st = sb.tile([128, NCH * N_PART], f32, tag="z_st")
            u_sb = sb.tile([128, RHO_PAD], f32r, tag="u_sb")
            v_sb = sb.tile([128, RHO_PAD], f32r, tag="v_sb")
            z_sb = sb.tile([128, NCH * N_PART], f32r, tag="z_sb")
            hprod = sb.tile([P, T * MS], f32, tag="hprod")
            earg = sb.tile([P, T * MS], f32, tag="earg")
            env = sb.tile([P, T * MS], f32, tag="env")
            basis = sb.tile([P, T * MS], f32, tag="basis")
            st = sb.tile([128, TG * TG], f32, tag="st")
            at_sb = sb.tile([128, TG * 128], f32r, tag="at_sb")
            bt_sb = sb.tile([128, TG * 128], f32r, tag="bt_sb")
            t_sb = sb.tile([128, NCH * G * (TG * P)], f32r, tag="t_sb")
            out_sb = sb.tile([N_PART, G * 2 * 448], f32, tag="out_sb")

            # ---- loads --------------------------------------------------
            # n-major hermite table: [P, (n:4)(t:32)(d:3)] so the x load and
            # all per-n writes are contiguous
            h4 = htab[:].rearrange("p (n t d) -> p n t d", n=4, t=T, d=SDIM)
            nc.sync.dma_start(h4[:, 1], x_d[:].rearrange(
                "p (t d) -> p t d", t=T, d=SDIM))
            nc.scalar.dma_start(s2c[:], s2c_d[:])
            nc.scalar.dma_start(ident[:], id_d[:])
            nc.gpsimd.dma_start(u_st[:], u_d[:])
            nc.gpsimd.dma_start(v_st[:], v_d[:])
            nc.scalar.dma_start(z_st[:], z_d[:])
            nc.scalar.copy(u_sb[:], u_st[:])
            nc.scalar.copy(v_sb[:], v_st[:])
            nc.scalar.copy(z_sb[:], z_st[:])

            # pad lanes of the packed transposed tensors must be finite
            nc.gpsimd.memset(at_sb[:].bitcast(f32), 0.0)
            nc.gpsimd.memset(bt_sb[:].bitcast(f32), 0.0)

            # ---- PE warm-up: ~20 junk matmuls so HAM reaches 8/8 before
            # the real matmul phase (PE is otherwise idle during basis build)
            wu_w = sb.tile([128, 128], dt.bfloat16, tag="wu_w")
            wu_r = sb.tile([128, 512], dt.bfloat16, tag="wu_r")
            wu_p = ps.tile([128, 512], f32, tag="pt")
            nc.gpsimd.memset(wu_w[:], 1.0)
            nc.gpsimd.memset(wu_r[:], 1.0)
            for wi in range(WARMUP_MM):
                nc.tensor.matmul(wu_p[:], wu_w[:], wu_r[:],
                                 start=(wi == 0), stop=(wi == WARMUP_MM - 1))

            # ---- hermite table -----------------------------------------
            # n=0 slot: ones
            nc.gpsimd.memset(h4[:, 0], 1.0)
            x_ap = h4[:, 1]
            nc.vector.tensor_tensor(x2[:].rearrange("p (t d) -> p t d", t=T, d=SDIM),
                                    x_ap, x_ap, op=Alu.mult)
            x2v = x2[:].rearrange("p (t d) -> p t d", t=T, d=SDIM)
            nc.vector.tensor_reduce(rho_t[:], x2v, axis=mybir.AxisListType.X,
                                    op=Alu.add)
            # h2' = x^2 - 0.5   (H2 = 4x^2-2 = 4*h2')
            nc.vector.tensor_scalar_sub(h4[:, 2], x2v, 0.5)
            # h3' = (x^2 - 1.5)*x   (H3 = 8x^3-12x = 8*h3')
            nc.vector.scalar_tensor_tensor(h4[:, 3], x2v, 1.5, x_ap,
                                           op0=Alu.subtract, op1=Alu.mult)

            # ---- pair products into hprod[:, :, 0:10] -------------------
            hp = hprod[:].rearrange("p (t m) -> p t m", t=T, m=MS)
            # mu0: (0,0) = 1
            nc.gpsimd.memset(hp[:, :, 0], 1.0)
            # mu1..3: H_{1..3}(x2)
            nc.scalar.copy(hp[:, :, 1:4], h4[:, 1:4, :, 2].transpose([0, 2, 1]))
            # mu4..6: h1(x1) * {1, h1(x2), h2'(x2)}
            x1h1 = h4[:, 1, :, 1].unsqueeze(-1).broadcast_to((P, T, 3))
            nc.vector.tensor_tensor(hp[:, :, 4:7], x1h1,
                                    h4[:, 0:3, :, 2].transpose([0, 2, 1]),
                                    op=Alu.mult)
            # mu7..8: h2'(x1) * {1, h1(x2)}
            x1h2 = h4[:, 2, :, 1].unsqueeze(-1).broadcast_to((P, T, 2))
            nc.vector.tensor_tensor(hp[:, :, 7:9], x1h2,
                                    h4[:, 0:2, :, 2].transpose([0, 2, 1]),
                                    op=Alu.mult)
            # mu9: h3'(x1)
            nc.scalar.copy(hp[:, :, 9], h4[:, 3, :, 1])

            # ---- x0 products into hprod[:, :, 10:20] --------------------
            x0h1 = h4[:, 1, :, 0].unsqueeze(-1)
            nc.vector.tensor_tensor(hp[:, :, 10:13],
                                    x0h1.broadcast_to((P, T, 3)),
                                    hp[:, :, 0:3], op=Alu.mult)
            nc.vector.tensor_tensor(hp[:, :, 13:15],
                                    x0h1.broadcast_to((P, T, 2)),
                                    hp[:, :, 4:6], op=Alu.mult)
            nc.vector.tensor_tensor(hp[:, :, 15], x0h1.squeeze(-1),
                                    hp[:, :, 7], op=Alu.mult)
            x0h2 = h4[:, 2, :, 0].unsqueeze(-1)
            nc.vector.tensor_tensor(hp[:, :, 16:18],
                                    x0h2.broadcast_to((P, T, 2)),
                                    hp[:, :, 0:2], op=Alu.mult)
            nc.vector.tensor_tensor(hp[:, :, 18], x0h2.squeeze(-1),
                                    hp[:, :, 4], op=Alu.mult)
            nc.scalar.copy(hp[:, :, 19], h4[:, 3, :, 0])

            # ---- envelope ----------------------------------------------
            ea = earg[:].rearrange("p (t m) -> p t m", t=T, m=MS)[:, :, 0:NB]
            nc.gpsimd.tensor_tensor(
                ea,
                rho_t[:].unsqueeze(-1).broadcast_to((P, T, NB)),
                s2c[:].unsqueeze(1).broadcast_to((P, T, NB)),
                op=Alu.mult)
            ev = env[:].rearrange("p (t m) -> p t m", t=T, m=MS)[:, :, 0:NB]
            bb = basis[:].rearrange("p (t m) -> p t m", t=T, m=MS)[:, :, 0:NB]
            hv = hprod[:].rearrange("p (t m) -> p t m", t=T, m=MS)[:, :, 0:NB]
            nc.scalar.activation(ev, ea, ActF.Exp)
            nc.vector.tensor_tensor(bb, hv, ev, op=Alu.mult)

            # ---- transpose into packed layout ---------------------------
            # chunk cc covers basis cols [128cc, 128cc+128) = tiles 4cc..4cc+3
            # transposed: btp[(jb:4)(mu:32), 112] with tile t = 4c + jb.
            # 4 chunks packed per PSUM tensor at 128-col spacing.
            btp0 = ps.tile([128, 512], f32, tag="pt")
            btp1 = ps.tile([128, 512], f32, tag="pt")
            btps = (btp0, btp1)
            for cc in range(8):
                nc.tensor.transpose(
                    btps[cc // 4][:, 128 * (cc % 4):128 * (cc % 4) + P],
                    basis[:, 128 * cc:128 * (cc + 1)],
                    ident[:],
                )


            # ---- S (segment sum over i) and A = S - B -------------------
            stv = st[:].rearrange("p (k c b) -> p k c b", k=2, c=4, b=TG)
            for k in range(2):
                bsrc = btps[k][:].rearrange(
                    "p (c s) -> p c s", c=4, s=128)[:, :, 0:P]
                bsrc_bi = bsrc.rearrange("p c (b i) -> p c b i", b=TG, i=N_PART)
                nc.vector.tensor_reduce(stv[:, k], bsrc_bi,
                                        axis=mybir.AxisListType.X, op=Alu.add)
                at_half = at_sb[:, 512 * k:512 * (k + 1)].rearrange(
                    "p (c s) -> p c s", c=4, s=128)[:, :, 0:P]
                at_bi = at_half.rearrange("p c (b i) -> p c b i", b=TG, i=N_PART)
                nc.vector.tensor_tensor(
                    at_bi,
                    stv[:, k].unsqueeze(-1).broadcast_to((128, 4, TG, N_PART)),
                    bsrc_bi, op=Alu.subtract)
                bt_half = bt_sb[:, 512 * k:512 * (k + 1)].rearrange(
                    "p (c s) -> p c s", c=4, s=128)[:, :, 0:P]
                nc.scalar.copy(bt_half, bsrc)

            # ---- rank projections + elementwise product -----------------
            tv = t_sb[:].rearrange("p (k r) -> p k r", k=NCH * G, r=TG * P)
            b_all = sb.tile([128, NCH * G * 1024], f32, tag="b_all")
            bav = b_all[:].rearrange("p (k s) -> p k s", k=NCH * G, s=1024)

            # phase B: all V-projections, dense PE; ACT drains PSUM -> SBUF
            for g in range(G):
                for c in range(NCH):
                    rk = c * G + g
                    b_ps = ps.tile([128, 1024], f32, tag="pt")
                    for h in range(2):
                        cs = slice(512 * h, 512 * (h + 1))
                        nc.tensor.matmul(
                            b_ps[:, cs],
                            v_sb[32 * g:32 * g + NB, 128 * c:128 * (c + 1)],
                            bt_sb[32 * g:32 * g + NB, cs],
                            start=True, stop=True, tile_position=(32 * g, 0))
                    bvv = b_ps[:].rearrange("p (j s) -> p j s", j=TG, s=128)[:, :, 0:P]
                    bsf = bav[:, rk].rearrange("p (j s) -> p j s", j=TG, s=128)[:, :, 0:P]
                    nc.scalar.copy(bsf, bvv)

            # phase A: U-projections + elementwise products (DVE)
            for g in range(G):
                for c in range(NCH):
                    rk = c * G + g
                    a_ps = ps.tile([128, 1024], f32, tag="pt")
                    for h in range(2):
                        cs = slice(512 * h, 512 * (h + 1))
                        nc.tensor.matmul(
                            a_ps[:, cs],
                            u_sb[32 * g:32 * g + NB, 128 * c:128 * (c + 1)],
                            at_sb[32 * g:32 * g + NB, cs],
                            start=True, stop=True, tile_position=(32 * g, 0))
                    av = a_ps[:].rearrange("p (j s) -> p j s", j=TG, s=128)[:, :, 0:P]
                    bsf = bav[:, rk].rearrange("p (j s) -> p j s", j=TG, s=128)[:, :, 0:P]
                    nc.vector.tensor_tensor(
                        tv[:, rk, :].rearrange("p (j q) -> p j q", j=TG, q=P),
                        av, bsf, op=Alu.mult)

            # ---- output projection: out^T = Z^T @ T ---------------------
            osv = out_sb[:].rearrange("p (j h s) -> p j h s", j=G, h=2, s=448)
            for j in range(G):
                o_ps = ps.tile([N_PART, 1024], f32, tag="pt")
                for h in range(2):
                    for c in range(NCH):
                        nc.tensor.matmul(
                            o_ps[0:N_PART, 512 * h:512 * h + 448],
                            z_sb[:, N_PART * c:N_PART * (c + 1)],
                            tv[:, c * G + j, 448 * h:448 * (h + 1)],
                            start=(c == 0), stop=(c == NCH - 1))
                ov = o_ps[:].rearrange("p (h s) -> p h s", h=2, s=512)[:, :, 0:448]
                if j % 2 == 0:
                    nc.vector.tensor_copy(osv[:, j], ov)
                else:
                    nc.scalar.copy(osv[:, j], ov)
            nc.sync.dma_start(out_d[:], out_sb[0:N_PART, :])

    nc.compile()
    return nc


def _get_program():
    global _PROGRAM
    if _PROGRAM is None:
        _PROGRAM = _build_program()
    return _PROGRAM


# ---------------------------------------------------------------------------
# entry point
# ---------------------------------------------------------------------------

LAST_RESULTS = None


def kernel(ri, rij_dist=None, sigma=None, coeff=None, **_unused):
    import os
    from concourse.bass_utils import run_bass_kernel_spmd

    global LAST_RESULTS
    ri = np.ascontiguousarray(np.asarray(ri, dtype=np.float32))
    U, V, Zd, s2c = _decompose(coeff, sigma)
    ident = np.eye(P, dtype=np.float32)

    nc = _get_program()
    in_maps = []
    for i in range(N_CORES):
        chunk = ri[i * BC:(i + 1) * BC].reshape(T, P, SDIM)
        x = np.ascontiguousarray(chunk.transpose(1, 0, 2).reshape(P, T * SDIM))
        in_maps.append({
            "x": x, "s2c": s2c, "ident": ident,
            "u": U, "v": V, "z": Zd,
        })

    trace = bool(int(os.environ.get("BOB_TRACE", "0")))
    res = run_bass_kernel_spmd(nc, in_maps, core_ids=list(range(N_CORES)),
                               trace=trace)
    LAST_RESULTS = res

    outs = []
    for i in range(N_CORES):
        ot = res.results[i]["out_t"]                      # (14, 3584)
        # device col order is (jb:4)(c:8)(p:112) with tile t = 4c + jb
        ot = ot.reshape(N_PART, G, 8, P).transpose(0, 2, 1, 3).reshape(N_PART, R)
        outs.append(ot.T.reshape(BC, N_PART, N_PART))
    return np.ascontiguousarray(np.concatenate(outs, axis=0), dtype=np.float32)

